# revision 15
# baseline (speedup 1.0000x reference)
"""Multi-head attention forward (B=2, N=2048, DIM=1024, H=16, D=64) on 8 TRN2
NeuronCores.

Sharding: 2-way data parallel over batch x 4-way tensor parallel over heads.
Core c: batch c//4, heads 4*(c%4) .. 4*(c%4)+3.

Per-core device kernel (all matmuls bf16, fp32 PSUM accumulation):
  1. QK projection into transposed layout qkT [feat(part), tok], head dims
     pre-permuted (even then odd per head) so RoPE pairs sit in partition
     blocks of 32.  Emitted per 512-token chunk with RoPE fused behind each
     chunk so attention unblocks as early as possible (Tile deps are
     range-based).
  2. RoPE: rot = qk*cos_rep + swap(qk)*sin_signed; swap is a partition block
     swap (SBUF->SBUF DMA), sin's sign folded host-side.
  3. V projection into natural [tok(part), d] layout with a ones column
     appended (softmax denominator rides the PV matmul).
  4. Attention, head-pair outer (row-packed K=64 S^T matmuls via
     tile_position), strips streamed back-to-back so the scalar engine's exp
     (the bottleneck: 16.8M elements at 1 elem/lane/cycle) never stalls:
     S^T[kj, qi] -> exp -> PV: lhsT=[V|1], rhs=expS^T -> psum [65, qi].
  5. Normalize: recip(denom row) -> gpsimd partition_broadcast -> multiply ->
     attnT [feat(part), tok] bf16.
  6. Output projection partial per strip: out_part[tok, 1024].
Host sums the 4 head-group partials per batch and adds the bias.
"""

import numpy as np
import ml_dtypes
from contextlib import ExitStack

import concourse.bass as bass
import concourse.mybir as mybir
import concourse.tile as tile
from concourse import bacc
from concourse import bass2jax

# problem constants (hardcoded per contract)
B, N, DIM, H, D = 2, 2048, 1024, 16, 64
HL = 4                      # heads per core
QKF = 2 * HL * D            # 512 qk features per core
VF = HL * D                 # 256 v features per core
SCALE = D ** -0.5
P = 128
KT = DIM // P               # 8 k tiles of the model dim
NKJ = N // P                # 16 key tiles
BF16 = mybir.dt.bfloat16
F32 = mybir.dt.float32
NPBF16 = ml_dtypes.bfloat16

_CACHE = {}


def _build_nc():
    nc = bacc.Bacc("TRN2", target_bir_lowering=False)

    xT = nc.declare_dram_parameter("xT", [DIM, N], BF16, isOutput=False)
    wqkT = nc.declare_dram_parameter("wqkT", [DIM, QKF], BF16, isOutput=False)
    wvT = nc.declare_dram_parameter("wvT", [DIM, VF], BF16, isOutput=False)
    wp = nc.declare_dram_parameter("wp", [VF, DIM], BF16, isOutput=False)
    cs = nc.declare_dram_parameter("cs", [P, N], BF16, isOutput=False)
    sn = nc.declare_dram_parameter("sn", [P, N], BF16, isOutput=False)
    out = nc.declare_dram_parameter("out", [N, DIM], F32, isOutput=True)

    xT_r = xT.rearrange("(k p) n -> p k n", p=P)
    wqkT_r = wqkT.rearrange("(k p) f -> p k f", p=P)
    wvT_r = wvT.rearrange("(k p) f -> p k f", p=P)
    wp_r = wp.rearrange("(k p) f -> p k f", p=P)
    out_r = out.rearrange("(m p) f -> m p f", p=P)

    with tile.TileContext(nc) as tc:
        with ExitStack() as ctx:
            singles = ctx.enter_context(tc.tile_pool(name="singles", bufs=1))
            # PSUM budget: st 2x[128,1024]=4 banks, pv 3x[128,512]=3, prj 1
            psum_st = ctx.enter_context(tc.tile_pool(name="psum_st", bufs=2, space="PSUM"))
            psum_pv = ctx.enter_context(tc.tile_pool(name="psum_pv", bufs=3, space="PSUM"))
            psum_prj = ctx.enter_context(tc.tile_pool(name="psum_prj", bufs=1, space="PSUM"))
            expp = ctx.enter_context(tc.tile_pool(name="expp", bufs=4))
            outp = ctx.enter_context(tc.tile_pool(name="outp", bufs=3))
            smallp = ctx.enter_context(tc.tile_pool(name="smallp", bufs=4))
            ropep = ctx.enter_context(tc.tile_pool(name="ropep", bufs=3))

            # ---- static loads (chunked per k-tile so compute starts early) ----
            xT_sb = singles.tile([P, KT, N], BF16, tag="xT_sb")
            wqkT_sb = singles.tile([P, KT, QKF], BF16, tag="wqkT_sb")
            wvT_sb = singles.tile([P, KT, VF], BF16, tag="wvT_sb")
            cs_sb = singles.tile([P, N], BF16, tag="cs_sb")
            sn_sb = singles.tile([P, N], BF16, tag="sn_sb")
            # critical path for the first exp: wqkT + cos/sin + xT token-block 0
            for m in (2, 0, 3, 1):
                nc.sync.dma_start(out=wqkT_sb[:, :, m * P:(m + 1) * P],
                                  in_=wqkT_r[:, :, m * P:(m + 1) * P])
            nc.sync.dma_start(out=cs_sb[:, 0:512], in_=cs[:, 0:512])
            nc.sync.dma_start(out=sn_sb[:, 0:512], in_=sn[:, 0:512])
            nc.sync.dma_start(out=xT_sb[:, :, 0:512], in_=xT_r[:, :, 0:512])
            nc.sync.dma_start(out=wvT_sb, in_=wvT_r)
            for b in range(1, 4):
                sl = slice(b * 512, (b + 1) * 512)
                nc.sync.dma_start(out=cs_sb[:, sl], in_=cs[:, sl])
                nc.sync.dma_start(out=sn_sb[:, sl], in_=sn[:, sl])
                nc.sync.dma_start(out=xT_sb[:, :, sl], in_=xT_r[:, :, sl])
            wp_sb = singles.tile([P, VF // P, DIM], BF16, tag="wp_sb")
            nc.sync.dma_start(out=wp_sb, in_=wp_r)

            qk_rot = singles.tile([P, 4, N], BF16, tag="qk_rot")
            vones = singles.tile([P, HL, NKJ, D + 1], BF16, tag="vones")
            attnT = singles.tile([P, VF // P, N], BF16, tag="attnT")
            attnT_odd = singles.tile([64, VF // P, N], BF16, tag="attnT_odd")

            # ---- QK projection chunk + fused RoPE -----------------------------
            def qk_chunk(m, t):
                sl = slice(t * 512, (t + 1) * 512)
                ps = psum_pv.tile([P, 512], F32, tag="pv", name=f"qk_{m}_{t}")
                for k in range(KT):
                    nc.tensor.matmul(
                        ps,
                        lhsT=wqkT_sb[:, k, m * P:(m + 1) * P],
                        rhs=xT_sb[:, k, sl],
                        start=(k == 0),
                        stop=(k == KT - 1),
                    )
                raw = ropep.tile([P, 512], BF16, tag="raw")
                nc.vector.tensor_copy(raw, ps)
                sw = ropep.tile([P, 512], BF16, tag="sw")
                for a in range(0, P, 64):
                    nc.sync.dma_start(out=sw[a:a + 32, :], in_=raw[a + 32:a + 64, :])
                    nc.sync.dma_start(out=sw[a + 32:a + 64, :], in_=raw[a:a + 32, :])
                t1 = ropep.tile([P, 512], BF16, tag="t1")
                nc.vector.tensor_mul(t1, raw, cs_sb[:, sl])
                t2 = ropep.tile([P, 512], BF16, tag="t2")
                nc.vector.tensor_mul(t2, sw, sn_sb[:, sl])
                nc.vector.tensor_add(qk_rot[:, m, sl], t1, t2)

            def v_chunk(t):
                ps = psum_pv.tile([P, 512], F32, tag="pv", name=f"v_{t}")
                for k in range(KT):
                    nc.tensor.matmul(
                        ps[:, :VF],
                        lhsT=xT_sb[:, k, t * P:(t + 1) * P],
                        rhs=wvT_sb[:, k, :],
                        start=(k == 0),
                        stop=(k == KT - 1),
                    )
                nc.vector.tensor_copy(vones[:, :, t, 0:D], ps[:, :VF])

            # pair 0 strip 0 consumes: all k chunks of tile 2, q chunk (0,0),
            # and all V chunks.  Emit exactly that prelude; defer the rest
            # between attention segments so PE fills exp-bound gaps with it.
            nc.vector.memset(vones[:, :, :, D:D + 1], 1.0)
            qk_chunk(2, 0)
            qk_chunk(0, 0)
            for t in range(NKJ):
                v_chunk(t)

            # filler schedules: deferred projection chunks emitted inside the
            # attention kj loops at a rate under the exp pace.  Key: (j, s).
            fillers = {
                (0, 0): {2: [("qk", 2, 1)], 6: [("qk", 2, 2)],
                         10: [("qk", 2, 3)], 13: [("qk", 0, 1)]},
                (0, 1): {2: [("qk", 3, 0)], 6: [("qk", 1, 0)],
                         10: [("qk", 0, 2)]},
                (0, 2): {2: [("qk", 3, 1)], 6: [("qk", 1, 1)],
                         10: [("qk", 0, 3)]},
                (0, 3): {2: [("qk", 3, 2)], 6: [("qk", 1, 2)],
                         10: [("qk", 3, 3)], 13: [("qk", 1, 3)]},
            }

            # ---- attention: pair outer, strips streamed -----------------------
            def norm_head(h, s, pvt):
                # custom DVE ops misbehave at base partition 64 on HW:
                # copy the denom row, DMA to partition 0, recip there
                dn = smallp.tile([D + 1, 512], F32, tag="dn")
                nc.vector.tensor_copy(dn[D:D + 1, :], pvt[D:D + 1, :])
                rci = smallp.tile([1, 512], F32, tag="rci")
                nc.sync.dma_start(out=rci, in_=dn[D:D + 1, :])
                rc = smallp.tile([1, 512], F32, tag="rc")
                nc.vector.reciprocal_approx_fast(out=rc, in_=rci)
                rb = smallp.tile([64, 512], F32, tag="rb")
                nc.gpsimd.partition_broadcast(rb, rc)
                kt = h // 2
                if h % 2 == 0:
                    dst = attnT[0:64, kt, s * 512:(s + 1) * 512]
                else:
                    dst = attnT_odd[0:64, kt, s * 512:(s + 1) * 512]
                nc.vector.tensor_mul(dst, pvt[0:D, :], rb)

            def shift_strip(s):
                nc.sync.dma_start(
                    out=attnT[64:P, :, s * 512:(s + 1) * 512],
                    in_=attnT_odd[:, :, s * 512:(s + 1) * 512],
                )

            def proj_chunk(mt, ch, last=False):
                if last:
                    ps = psum_st.tile([P, 1024], F32, tag="st",
                                      name=f"prj_{mt}_{ch}")[:, 0:512]
                else:
                    ps = psum_prj.tile([P, 512], F32, tag="prj",
                                       name=f"prj_{mt}_{ch}")
                for kt in range(VF // P):
                    nc.tensor.matmul(
                        ps,
                        lhsT=attnT[:, kt, mt * P:(mt + 1) * P],
                        rhs=wp_sb[:, kt, ch * 512:(ch + 1) * 512],
                        start=(kt == 0),
                        stop=(kt == VF // P - 1),
                    )
                ob = outp.tile([P, 512], F32, tag="ob")
                nc.vector.tensor_copy(ob, ps)
                nc.sync.dma_start(out=out_r[mt, :, ch * 512:(ch + 1) * 512], in_=ob)

            for j in range(2):          # head pairs (2j, 2j+1)
                for s in range(4):      # qi strips of 512, streamed
                    pv = [psum_pv.tile([P, 512], F32, tag="pv",
                                       name=f"pv_{j}_{s}_{i}")
                          for i in range(2)]
                    for kj in range(NKJ):
                        for f in fillers.get((j, s), {}).get(kj, ()):
                            if f[0] == "qk":
                                qk_chunk(f[1], f[2])
                            else:
                                v_chunk(f[1])
                        if j == 1 and s > 0 and kj >= 1 and kj % 2 == 1:
                            proj_chunk(4 * (s - 1) + (kj - 1) // 4,
                                       ((kj - 1) // 2) % 2)
                        st = psum_st.tile([P, 1024], F32, tag="st",
                                          name=f"st_{j}_{s}_{kj}")
                        nc.tensor.matmul(
                            st[:, 0:512],
                            lhsT=qk_rot[0:64, 2 + j, kj * P:(kj + 1) * P],
                            rhs=qk_rot[0:64, j, s * 512:(s + 1) * 512],
                            start=True, stop=True,
                            tile_position=(0, 0),
                        )
                        nc.tensor.matmul(
                            st[:, 512:1024],
                            lhsT=qk_rot[64:P, 2 + j, kj * P:(kj + 1) * P],
                            rhs=qk_rot[64:P, j, s * 512:(s + 1) * 512],
                            start=True, stop=True,
                            tile_position=(64, 0),
                        )
                        es = expp.tile([P, 1024], BF16, tag="expS")
                        nc.scalar.activation(
                            es, st, mybir.ActivationFunctionType.Exp, scale=SCALE
                        )
                        for hh in range(2):
                            nc.tensor.matmul(
                                pv[hh][0:D + 1, :],
                                lhsT=vones[:, 2 * j + hh, kj, :],
                                rhs=es[:, hh * 512:(hh + 1) * 512],
                                start=(kj == 0),
                                stop=(kj == NKJ - 1),
                            )
                    for hh in range(2):
                        norm_head(2 * j + hh, s, pv[hh])
                    if j == 1:
                        shift_strip(s)
            # tail: last strip's projection (st pool is free by now)
            for mt in range(12, 16):
                for ch in range(2):
                    proj_chunk(mt, ch, last=True)

    nc.compile()
    return nc


def _make_in_maps(x, freqs, w_qkv, w_proj):
    # RoPE even/odd permutation of q/k head dims (host side, free)
    evens = np.arange(0, D, 2)
    odds = np.arange(1, D, 2)
    perm64 = np.concatenate([evens, odds])
    permH = np.concatenate([h * D + perm64 for h in range(HL)])

    wq = w_qkv[0:DIM]
    wk = w_qkv[DIM:2 * DIM]
    wv = w_qkv[2 * DIM:3 * DIM]

    cos = np.cos(freqs).astype(np.float32)   # [N, 32]
    sin = np.sin(freqs).astype(np.float32)
    pidx = np.arange(P) % 32
    cs_rep = cos[:, pidx].T.copy()           # [128, N]
    sgn = np.where((np.arange(P) % 64) < 32, -1.0, 1.0).astype(np.float32)
    sn_rep = (sin[:, pidx] * sgn[None, :]).T.copy()
    cs_b = cs_rep.astype(NPBF16)
    sn_b = sn_rep.astype(NPBF16)

    in_maps = []
    for c in range(8):
        b, g = c // 4, c % 4
        rows = slice(g * VF, (g + 1) * VF)
        wq_p = wq[rows][permH]               # [256, 1024]
        wk_p = wk[rows][permH]
        wqkT = np.concatenate([wq_p, wk_p], axis=0).T.copy()   # [1024, 512]
        wvT = wv[rows].T.copy()                                # [1024, 256]
        wp_rhs = w_proj[:, rows].T.copy()                      # [256, 1024]
        xT = x[b].T.copy()                                     # [1024, 2048]
        in_maps.append({
            "xT": xT.astype(NPBF16),
            "wqkT": wqkT.astype(NPBF16),
            "wvT": wvT.astype(NPBF16),
            "wp": wp_rhs.astype(NPBF16),
            "cs": cs_b,
            "sn": sn_b,
        })
    return in_maps


def _get_nc():
    if "nc" not in _CACHE:
        _CACHE["nc"] = _build_nc()
    return _CACHE["nc"]


def kernel(x, freqs, w_qkv, w_proj, b_proj):
    x = np.asarray(x, dtype=np.float32)
    freqs = np.asarray(freqs, dtype=np.float32)
    w_qkv = np.asarray(w_qkv, dtype=np.float32)
    w_proj = np.asarray(w_proj, dtype=np.float32)
    b_proj = np.asarray(b_proj, dtype=np.float32)

    nc = _get_nc()
    in_maps = _make_in_maps(x, freqs, w_qkv, w_proj)
    results = bass2jax.run_bass_via_pjrt(nc, in_maps, n_cores=8)

    out = np.zeros((B, N, DIM), dtype=np.float32)
    for c in range(8):
        out[c // 4] += np.asarray(results[c]["out"], dtype=np.float32)
    out += b_proj[None, None, :]
    return out


# revision 17
# speedup vs baseline: 1.0049x; 1.0049x over previous
"""Multi-head attention forward (B=2, N=2048, DIM=1024, H=16, D=64) on 8 TRN2
NeuronCores.

Sharding: 2-way data parallel over batch x 4-way tensor parallel over heads.
Core c: batch c//4, heads 4*(c%4) .. 4*(c%4)+3.

Per-core device kernel (all matmuls bf16, fp32 PSUM accumulation):
  1. QK projection into transposed layout qkT [feat(part), tok], head dims
     pre-permuted (even then odd per head) so RoPE pairs sit in partition
     blocks of 32.  Emitted per 512-token chunk with RoPE fused behind each
     chunk so attention unblocks as early as possible (Tile deps are
     range-based).
  2. RoPE: rot = qk*cos_rep + swap(qk)*sin_signed; swap is a partition block
     swap (SBUF->SBUF DMA), sin's sign folded host-side.
  3. V projection into natural [tok(part), d] layout with a ones column
     appended (softmax denominator rides the PV matmul).
  4. Attention, head-pair outer (row-packed K=64 S^T matmuls via
     tile_position), strips streamed back-to-back so the scalar engine's exp
     (the bottleneck: 16.8M elements at 1 elem/lane/cycle) never stalls:
     S^T[kj, qi] -> exp -> PV: lhsT=[V|1], rhs=expS^T -> psum [65, qi].
  5. Normalize: recip(denom row) -> gpsimd partition_broadcast -> multiply ->
     attnT [feat(part), tok] bf16.
  6. Output projection partial per strip: out_part[tok, 1024].
Host sums the 4 head-group partials per batch and adds the bias.
"""

import numpy as np
import ml_dtypes
from contextlib import ExitStack

import concourse.bass as bass
import concourse.mybir as mybir
import concourse.tile as tile
from concourse import bacc
from concourse import bass2jax

# problem constants (hardcoded per contract)
B, N, DIM, H, D = 2, 2048, 1024, 16, 64
HL = 4                      # heads per core
QKF = 2 * HL * D            # 512 qk features per core
VF = HL * D                 # 256 v features per core
SCALE = D ** -0.5
P = 128
KT = DIM // P               # 8 k tiles of the model dim
NKJ = N // P                # 16 key tiles
BF16 = mybir.dt.bfloat16
F32 = mybir.dt.float32
NPBF16 = ml_dtypes.bfloat16

_CACHE = {}


def _build_nc():
    nc = bacc.Bacc("TRN2", target_bir_lowering=False)

    xT = nc.declare_dram_parameter("xT", [DIM, N], BF16, isOutput=False)
    wqkT = nc.declare_dram_parameter("wqkT", [DIM, QKF], BF16, isOutput=False)
    wvT = nc.declare_dram_parameter("wvT", [DIM, VF], BF16, isOutput=False)
    wp = nc.declare_dram_parameter("wp", [VF, DIM], BF16, isOutput=False)
    cs = nc.declare_dram_parameter("cs", [P, N], BF16, isOutput=False)
    sn = nc.declare_dram_parameter("sn", [P, N], BF16, isOutput=False)
    out = nc.declare_dram_parameter("out", [N, DIM], F32, isOutput=True)

    xT_r = xT.rearrange("(k p) n -> p k n", p=P)
    wqkT_r = wqkT.rearrange("(k p) f -> p k f", p=P)
    wvT_r = wvT.rearrange("(k p) f -> p k f", p=P)
    wp_r = wp.rearrange("(k p) f -> p k f", p=P)
    out_r = out.rearrange("(m p) f -> m p f", p=P)

    with tile.TileContext(nc) as tc:
        with ExitStack() as ctx:
            singles = ctx.enter_context(tc.tile_pool(name="singles", bufs=1))
            # PSUM budget: st 2x[128,1024]=4 banks, pv 3x[128,512]=3, prj 1
            psum_st = ctx.enter_context(tc.tile_pool(name="psum_st", bufs=2, space="PSUM"))
            psum_pv = ctx.enter_context(tc.tile_pool(name="psum_pv", bufs=3, space="PSUM"))
            psum_prj = ctx.enter_context(tc.tile_pool(name="psum_prj", bufs=1, space="PSUM"))
            expp = ctx.enter_context(tc.tile_pool(name="expp", bufs=4))
            outp = ctx.enter_context(tc.tile_pool(name="outp", bufs=3))
            smallp = ctx.enter_context(tc.tile_pool(name="smallp", bufs=4))
            ropep = ctx.enter_context(tc.tile_pool(name="ropep", bufs=3))

            # ---- static loads (chunked per k-tile so compute starts early) ----
            xT_sb = singles.tile([P, KT, N], BF16, tag="xT_sb")
            wqkT_sb = singles.tile([P, KT, QKF], BF16, tag="wqkT_sb")
            wvT_sb = singles.tile([P, KT, VF], BF16, tag="wvT_sb")
            cs_sb = singles.tile([P, N], BF16, tag="cs_sb")
            sn_sb = singles.tile([P, N], BF16, tag="sn_sb")
            # critical path for the first exp: wqkT + cos/sin + xT token-block 0
            for m in (2, 0, 3, 1):
                nc.sync.dma_start(out=wqkT_sb[:, :, m * P:(m + 1) * P],
                                  in_=wqkT_r[:, :, m * P:(m + 1) * P])
            nc.sync.dma_start(out=cs_sb[:, 0:512], in_=cs[:, 0:512])
            nc.sync.dma_start(out=sn_sb[:, 0:512], in_=sn[:, 0:512])
            nc.sync.dma_start(out=xT_sb[:, :, 0:512], in_=xT_r[:, :, 0:512])
            nc.sync.dma_start(out=wvT_sb, in_=wvT_r)
            for b in range(1, 4):
                sl = slice(b * 512, (b + 1) * 512)
                nc.sync.dma_start(out=cs_sb[:, sl], in_=cs[:, sl])
                nc.sync.dma_start(out=sn_sb[:, sl], in_=sn[:, sl])
                nc.sync.dma_start(out=xT_sb[:, :, sl], in_=xT_r[:, :, sl])
            wp_sb = singles.tile([P, VF // P, DIM], BF16, tag="wp_sb")
            nc.sync.dma_start(out=wp_sb, in_=wp_r)

            qk_rot = singles.tile([P, 4, N], BF16, tag="qk_rot")
            vones = singles.tile([P, HL, NKJ, D + 1], BF16, tag="vones")
            attnT = singles.tile([P, VF // P, N], BF16, tag="attnT")
            attnT_odd = singles.tile([64, VF // P, N], BF16, tag="attnT_odd")

            # ---- QK projection chunk + fused RoPE -----------------------------
            def qk_chunk(m, t):
                sl = slice(t * 512, (t + 1) * 512)
                ps = psum_pv.tile([P, 512], F32, tag="pv", name=f"qk_{m}_{t}")
                for k in range(KT):
                    nc.tensor.matmul(
                        ps,
                        lhsT=wqkT_sb[:, k, m * P:(m + 1) * P],
                        rhs=xT_sb[:, k, sl],
                        start=(k == 0),
                        stop=(k == KT - 1),
                    )
                raw = ropep.tile([P, 512], BF16, tag="raw")
                nc.vector.tensor_copy(raw, ps)
                sw = ropep.tile([P, 512], BF16, tag="sw")
                for a in range(0, P, 64):
                    nc.sync.dma_start(out=sw[a:a + 32, :], in_=raw[a + 32:a + 64, :])
                    nc.sync.dma_start(out=sw[a + 32:a + 64, :], in_=raw[a:a + 32, :])
                t1 = ropep.tile([P, 512], BF16, tag="t1")
                nc.vector.tensor_mul(t1, raw, cs_sb[:, sl])
                t2 = ropep.tile([P, 512], BF16, tag="t2")
                nc.vector.tensor_mul(t2, sw, sn_sb[:, sl])
                nc.vector.tensor_add(qk_rot[:, m, sl], t1, t2)

            def v_chunk(t):
                ps = psum_pv.tile([P, 512], F32, tag="pv", name=f"v_{t}")
                for k in range(KT):
                    nc.tensor.matmul(
                        ps[:, :VF],
                        lhsT=xT_sb[:, k, t * P:(t + 1) * P],
                        rhs=wvT_sb[:, k, :],
                        start=(k == 0),
                        stop=(k == KT - 1),
                    )
                nc.vector.tensor_copy(vones[:, :, t, 0:D], ps[:, :VF])

            # pair 0 strip 0 consumes: all k chunks of tile 2, q chunk (0,0),
            # and all V chunks.  Emit exactly that prelude; defer the rest
            # between attention segments so PE fills exp-bound gaps with it.
            nc.vector.memset(vones[:, :, :, D:D + 1], 1.0)
            qk_chunk(2, 0)
            qk_chunk(0, 0)
            for t in range(NKJ):
                v_chunk(t)

            # filler schedules: deferred projection chunks emitted inside the
            # attention kj loops at a rate under the exp pace.  Key: (j, s).
            fillers = {
                (0, 0): {2: [("qk", 2, 1)], 6: [("qk", 2, 2)],
                         10: [("qk", 2, 3)], 13: [("qk", 0, 1)]},
                (0, 1): {2: [("qk", 3, 0)], 6: [("qk", 1, 0)],
                         10: [("qk", 0, 2)]},
                (0, 2): {2: [("qk", 3, 1)], 6: [("qk", 1, 1)],
                         10: [("qk", 0, 3)]},
                (0, 3): {2: [("qk", 3, 2)], 6: [("qk", 1, 2)],
                         10: [("qk", 3, 3)], 13: [("qk", 1, 3)]},
            }

            # ---- attention: pair outer, strips streamed -----------------------
            def norm_head(h, s, pvt):
                # custom DVE ops misbehave at base partition 64 on HW:
                # copy the denom row, DMA to partition 0, recip there
                dn = smallp.tile([D + 1, 512], F32, tag="dn")
                nc.vector.tensor_copy(dn[D:D + 1, :], pvt[D:D + 1, :])
                rci = smallp.tile([1, 512], F32, tag="rci")
                nc.sync.dma_start(out=rci, in_=dn[D:D + 1, :])
                rc = smallp.tile([1, 512], F32, tag="rc")
                nc.vector.reciprocal_approx_fast(out=rc, in_=rci)
                rb = smallp.tile([64, 512], F32, tag="rb")
                nc.gpsimd.partition_broadcast(rb, rc)
                kt = h // 2
                if h % 2 == 0:
                    dst = attnT[0:64, kt, s * 512:(s + 1) * 512]
                else:
                    dst = attnT_odd[0:64, kt, s * 512:(s + 1) * 512]
                nc.vector.tensor_mul(dst, pvt[0:D, :], rb)

            def shift_strip(s):
                nc.sync.dma_start(
                    out=attnT[64:P, :, s * 512:(s + 1) * 512],
                    in_=attnT_odd[:, :, s * 512:(s + 1) * 512],
                )

            def proj_chunk(mt, ch, last=False):
                if last:
                    ps = psum_st.tile([P, 1024], F32, tag="st",
                                      name=f"prj_{mt}_{ch}")[:, 0:512]
                else:
                    ps = psum_prj.tile([P, 512], F32, tag="prj",
                                       name=f"prj_{mt}_{ch}")
                for kt in range(VF // P):
                    nc.tensor.matmul(
                        ps,
                        lhsT=attnT[:, kt, mt * P:(mt + 1) * P],
                        rhs=wp_sb[:, kt, ch * 512:(ch + 1) * 512],
                        start=(kt == 0),
                        stop=(kt == VF // P - 1),
                    )
                ob = outp.tile([P, 512], F32, tag="ob")
                nc.vector.tensor_copy(ob, ps)
                nc.sync.dma_start(out=out_r[mt, :, ch * 512:(ch + 1) * 512], in_=ob)

            for j in range(2):          # head pairs (2j, 2j+1)
                for s in range(4):      # qi strips of 512, streamed
                    pv = [psum_pv.tile([P, 512], F32, tag="pv",
                                       name=f"pv_{j}_{s}_{i}")
                          for i in range(2)]
                    for kj in range(NKJ):
                        for f in fillers.get((j, s), {}).get(kj, ()):
                            if f[0] == "qk":
                                qk_chunk(f[1], f[2])
                            else:
                                v_chunk(f[1])
                        if j == 1 and s > 0 and kj >= 1 and kj % 2 == 1:
                            proj_chunk(4 * (s - 1) + (kj - 1) // 4,
                                       ((kj - 1) // 2) % 2)
                        st = psum_st.tile([P, 1024], F32, tag="st",
                                          name=f"st_{j}_{s}_{kj}")
                        nc.tensor.matmul(
                            st[:, 0:512],
                            lhsT=qk_rot[0:64, 2 + j, kj * P:(kj + 1) * P],
                            rhs=qk_rot[0:64, j, s * 512:(s + 1) * 512],
                            start=True, stop=True,
                            tile_position=(0, 0),
                        )
                        nc.tensor.matmul(
                            st[:, 512:1024],
                            lhsT=qk_rot[64:P, 2 + j, kj * P:(kj + 1) * P],
                            rhs=qk_rot[64:P, j, s * 512:(s + 1) * 512],
                            start=True, stop=True,
                            tile_position=(64, 0),
                        )
                        es = expp.tile([P, 1024], BF16, tag="expS")
                        nc.scalar.activation(
                            es, st, mybir.ActivationFunctionType.Exp, scale=SCALE
                        )
                        for hh in range(2):
                            nc.tensor.matmul(
                                pv[hh][0:D + 1, :],
                                lhsT=vones[:, 2 * j + hh, kj, :],
                                rhs=es[:, hh * 512:(hh + 1) * 512],
                                start=(kj == 0),
                                stop=(kj == NKJ - 1),
                            )
                    for hh in range(2):
                        norm_head(2 * j + hh, s, pv[hh])
                    if j == 1:
                        shift_strip(s)
            # tail: last strip's projection (st pool is free by now)
            for mt in range(12, 16):
                for ch in range(2):
                    proj_chunk(mt, ch, last=True)

    nc.compile()
    return nc


def _make_in_maps(x, freqs, w_qkv, w_proj):
    # RoPE even/odd permutation of q/k head dims (host side, free)
    evens = np.arange(0, D, 2)
    odds = np.arange(1, D, 2)
    perm64 = np.concatenate([evens, odds])
    permH = np.concatenate([h * D + perm64 for h in range(HL)])

    wq = w_qkv[0:DIM]
    wk = w_qkv[DIM:2 * DIM]
    wv = w_qkv[2 * DIM:3 * DIM]

    cos = np.cos(freqs).astype(np.float32)   # [N, 32]
    sin = np.sin(freqs).astype(np.float32)
    pidx = np.arange(P) % 32
    cs_rep = cos[:, pidx].T.copy()           # [128, N]
    sgn = np.where((np.arange(P) % 64) < 32, -1.0, 1.0).astype(np.float32)
    sn_rep = (sin[:, pidx] * sgn[None, :]).T.copy()
    cs_b = cs_rep.astype(NPBF16)
    sn_b = sn_rep.astype(NPBF16)

    in_maps = []
    for c in range(8):
        b, g = c // 4, c % 4
        rows = slice(g * VF, (g + 1) * VF)
        wq_p = wq[rows][permH]               # [256, 1024]
        wk_p = wk[rows][permH]
        wqkT = np.concatenate([wq_p, wk_p], axis=0).T.copy()   # [1024, 512]
        wvT = wv[rows].T.copy()                                # [1024, 256]
        wp_rhs = w_proj[:, rows].T.copy()                      # [256, 1024]
        xT = x[b].T.copy()                                     # [1024, 2048]
        in_maps.append({
            "xT": xT.astype(NPBF16),
            "wqkT": wqkT.astype(NPBF16),
            "wvT": wvT.astype(NPBF16),
            "wp": wp_rhs.astype(NPBF16),
            "cs": cs_b,
            "sn": sn_b,
        })
    return in_maps


def _reset_device():
    try:
        import ctypes
        import jax
        jax.devices()
        lib = ctypes.CDLL("/opt/axon/libaxon_pjrt.so")
        if hasattr(lib, "axon_reset"):
            lib.axon_reset.restype = ctypes.c_int64
            lib.axon_reset()
    except Exception:
        pass


def _get_nc():
    if "nc" not in _CACHE:
        _CACHE["nc"] = _build_nc()
    return _CACHE["nc"]


def kernel(x, freqs, w_qkv, w_proj, b_proj):
    x = np.asarray(x, dtype=np.float32)
    freqs = np.asarray(freqs, dtype=np.float32)
    w_qkv = np.asarray(w_qkv, dtype=np.float32)
    w_proj = np.asarray(w_proj, dtype=np.float32)
    b_proj = np.asarray(b_proj, dtype=np.float32)

    nc = _get_nc()
    in_maps = _make_in_maps(x, freqs, w_qkv, w_proj)
    try:
        results = bass2jax.run_bass_via_pjrt(nc, in_maps, n_cores=8)
    except Exception:
        # a previously crashed run can leave the accelerator unrecoverable;
        # reset once and retry
        _reset_device()
        results = bass2jax.run_bass_via_pjrt(nc, in_maps, n_cores=8)

    out = np.zeros((B, N, DIM), dtype=np.float32)
    for c in range(8):
        out[c // 4] += np.asarray(results[c]["out"], dtype=np.float32)
    out += b_proj[None, None, :]
    return out
